# revision 1
# baseline (speedup 1.0000x reference)
"""Trainium2 Bass kernel for a custom Jacobi-basis layer.

Math:
    t = tanh(x)                                  x: [B, I] f32
    J[b,i,k] = P_k^(1,1)(t[b,i])                 Jacobi polys, k = 0..8
    out[b,o] = sum_{i,k} J[b,i,k] * coeff[o,i,k] * weights[o,i]

Strategy (8 NeuronCores, data-parallel over batch):
  * Fold weights into coeff on host: Cw[o,i,k] = coeff[o,i,k]*weights[o,i].
  * alpha=beta=1 makes the three-term recurrence two-term coefficient-free
    after rescaling: G_1 = t, G_k = t*G_{k-1} - B'_k*G_{k-2} with G_k = c_k*J_k.
    The 1/c_k scale is folded into the (host-prepared) matmul operand.
  * J_0 == 1, so the k=0 term is a per-output bias, applied with a K=1 matmul.
  * Per core: tanh/square on ScalarE, fp32 recurrence on VectorE (fused
    scalar_tensor_tensor ops, L/R half chains), one fp16 rounding cast per
    plane chunk on ScalarE, then 128 fp16 matmuls [128x128]@[128x512]
    accumulating fp32 in PSUM over the 4096-long (i,k) contraction.
    fp16 matmul error is ~3e-4 (vs 2.3e-3 bf16) and runs at full PE rate.
    Keeping the recurrence itself in fp32 avoids error compounding over k
    (a bf16 recurrence measures 2e-2; this pipeline measures ~3.6e-4).
  * DMA priority ladder: consts -> xt halves -> r planes (depth-2), so the
    tanh/recurrence/matmul pipeline starts as early as possible; PE is
    HAM-warmed with memset-sourced junk matmuls before the real stream.
"""

import numpy as np

import concourse.mybir as mybir
import concourse.tile as tile
from concourse import bacc
from concourse.bass_utils import run_bass_kernel_spmd

ORDER = 8
B, I, O = 4096, 512, 512
NCORES = 8
BC = B // NCORES          # batch rows per core = 512
P = 128                   # partitions
NIC = I // P              # i-chunks = 4
BT = BC // P              # b-tiles per core = 4
FREE = NIC * BC           # free dim of basis planes = 2048


def _consts():
    """Recurrence constants (alpha=beta=1, so the k2 term is 0)."""
    a = b = 1.0
    A, Bk = {}, {}
    for i in range(2, ORDER + 1):
        A[i] = (2 * i + a + b) * (2 * i + a + b - 1) / (2 * i * (i + a + b))
        Bk[i] = (i + a - 1) * (i + b - 1) * (2 * i + a + b) / (
            i * (i + a + b) * (2 * i + a + b - 2)
        )
    c = {0: 1.0, 1: 0.5}
    for i in range(2, ORDER + 1):
        c[i] = c[i - 1] / A[i]
    Bp = {i: Bk[i] * c[i] / c[i - 2] for i in range(2, ORDER + 1)}
    return c, Bp


def _build_module():
    nc = bacc.Bacc("TRN2", num_devices=NCORES)
    f32 = mybir.dt.float32
    f16 = mybir.dt.float16

    # xt stored half-major: [h, p, H] so each half is one contiguous DMA
    xt_d = nc.dram_tensor("xt", [2, P, FREE // 2], f32, kind="ExternalInput")
    # r layout: [p, (k-1)*FREE + ic*O + o] = Cw[o, ic*128+p, k] / c_k
    r_d = nc.dram_tensor("r", [P, ORDER * FREE], f16, kind="ExternalInput")
    # consts row 0 = [ones(128) | bias(512)]; rows 1..127 warmup junk
    consts_d = nc.dram_tensor("consts", [P, P + O], f16, kind="ExternalInput")
    # out layout: [p, bt*O + o] = output[core*BC + bt*128 + p, o]
    out_d = nc.dram_tensor("out", [P, BT * O], f32, kind="ExternalOutput")

    _, Bp = _consts()
    mult = mybir.AluOpType.mult
    add = mybir.AluOpType.add

    from concourse.tile_rust import add_dep_helper

    with tile.TileContext(nc) as tc:
        with (
            tc.tile_pool(name="io", bufs=1) as io,
            tc.tile_pool(name="g", bufs=1) as gp,
            tc.tile_pool(name="u", bufs=2) as up,
            tc.tile_pool(name="psum", bufs=1, space="PSUM") as pp,
        ):
            def chunk(ap, ic):
                return ap[:, ic * BC : (ic + 1) * BC]

            # consts first (tiny; also feeds the PE warmup), then xt in four
            # chained ic-chunks, then the r planes on a depth-2 ladder.
            const_t = io.tile([P, P + O], f16, tag="consts")
            nc.sync.dma_start(const_t[:], consts_d[:])
            ones_t = const_t[0:1, 0:P]
            bias_t = const_t[0:1, P : P + O]
            x_t = io.tile([P, FREE], f32, tag="x")
            H = FREE // 2
            d_xl = nc.sync.dma_start(x_t[:, 0:H], xt_d[0])
            d_prev_x = nc.sync.dma_start(x_t[:, H:FREE], xt_d[1])
            add_dep_helper(d_prev_x.ins, d_xl.ins, reason="dma ladder")
            # r planes ladder behind the xt halves (xt gates the whole
            # compute pipeline; r_k is only needed when PE reaches plane k).
            r_t = []
            d_prev = [None, d_prev_x]
            for k in range(ORDER):
                rt = io.tile([P, FREE], f16, tag=f"r{k}", name=f"r{k}")
                d = nc.sync.dma_start(rt[:], r_d[:, k * FREE : (k + 1) * FREE])
                if d_prev[k % 2] is not None:
                    add_dep_helper(d.ins, d_prev[k % 2].ins, reason="dma ladder")
                d_prev[k % 2] = d
                r_t.append(rt)

            # Basis planes G_1..G_8: recurrence in fp32 on VectorE at per-ic
            # granularity (the four ic-chunks are independent chains), each
            # chunk rounded to fp16 on ScalarE for the matmuls. G_8 is written
            # in fp16 directly (nothing downstream needs it in fp32).
            g = [None] * (ORDER + 1)
            gr = [None] * (ORDER + 1)

            t = gp.tile([P, FREE], f32, tag="t")
            sq = up.tile([P, FREE], f32, tag="sq")
            gr[1] = gp.tile([P, FREE], f16, tag="gr", name="gr1", bufs=4)
            for ic in range(NIC):
                nc.scalar.activation(
                    chunk(t, ic), chunk(x_t, ic),
                    mybir.ActivationFunctionType.Tanh,
                )
                nc.scalar.square(chunk(sq, ic), chunk(t, ic))
                nc.scalar.copy(chunk(gr[1], ic), chunk(t, ic))
            g[1] = t
            # g2 = s - B2 on ScalarE (off the DVE chain)
            g2 = gp.tile([P, FREE], f32, tag="g", name="g2", bufs=3)
            gr[2] = gp.tile([P, FREE], f16, tag="gr", name="gr2", bufs=4)
            for ic in range(NIC):
                nc.scalar.activation(
                    chunk(g2, ic), chunk(sq, ic),
                    mybir.ActivationFunctionType.Copy, bias=-Bp[2],
                )
                nc.scalar.copy(chunk(gr[2], ic), chunk(g2, ic))
            g[2] = g2

            # DVE chain at L/R half granularity (lower per-op overhead; the
            # two halves are independent chains). u3 = (s - B2)*t skips g2.
            halves = (slice(0, H), slice(H, FREE))
            u3 = up.tile([P, FREE], f32, tag="u", name="u3")
            g3 = gp.tile([P, FREE], f32, tag="g", name="g3", bufs=3)
            gr[3] = gp.tile([P, FREE], f16, tag="gr", name="gr3", bufs=4)
            for h in (0, 1):
                sl = halves[h]
                nc.vector.scalar_tensor_tensor(
                    u3[:, sl], sq[:, sl], -Bp[2], t[:, sl], add, mult
                )
            for h in (0, 1):
                sl = halves[h]
                nc.vector.scalar_tensor_tensor(
                    g3[:, sl], t[:, sl], -Bp[3], u3[:, sl], mult, add
                )
                for ic in (0, 1) if h == 0 else (2, 3):
                    nc.scalar.copy(chunk(gr[3], ic), chunk(g3, ic))
            g[3] = g3
            for k in range(4, ORDER + 1):
                u = up.tile([P, FREE], f32, tag="u", name=f"u{k}")
                last = k == ORDER
                gk = (
                    gp.tile([P, FREE], f16, tag="gr", name=f"g{k}", bufs=4)
                    if last
                    else gp.tile([P, FREE], f32, tag="g", name=f"g{k}", bufs=3)
                )
                if not last:
                    gr[k] = gp.tile(
                        [P, FREE], f16, tag="gr", name=f"gr{k}", bufs=4
                    )
                for h in (0, 1):
                    sl = halves[h]
                    nc.vector.tensor_tensor(
                        u[:, sl], t[:, sl], g[k - 1][:, sl], mult
                    )
                for h in (0, 1):
                    sl = halves[h]
                    nc.vector.scalar_tensor_tensor(
                        gk[:, sl], g[k - 2][:, sl], -Bp[k], u[:, sl], mult, add
                    )
                    if not last:
                        for ic in (0, 1) if h == 0 else (2, 3):
                            nc.scalar.copy(chunk(gr[k], ic), chunk(gk, ic))
                g[k] = gk
                if last:
                    gr[k] = gk

            # Matmuls: psum[bt] = ones^T @ bias + sum_{k,ic} G_k_slice^T @ R_k_slice
            psums = [
                pp.tile([P, O], f32, tag=f"ps{bt}", name=f"ps{bt}")
                for bt in range(BT)
            ]
            # HAM warmup with real K=128 matmuls on the consts block so the
            # clock gate is released before the real stream begins.
            ps_warm = pp.tile([P, O], f32, tag="warm", name="ps_warm")
            warm_t = io.tile([P, P + O], f16, tag="warmsrc")
            nc.vector.memset(warm_t[:], 0.25)
            for w in range(10):
                nc.tensor.matmul(
                    ps_warm[:],
                    warm_t[:, 0:P],
                    warm_t[:, P : P + O],
                    start=True,
                    stop=True,
                )
            for bt in range(BT):
                nc.tensor.matmul(
                    psums[bt][:], ones_t, bias_t, start=True, stop=False
                )
            out_t = io.tile([P, BT * O], f32, tag="out")
            for k in range(1, ORDER + 1):
                if k < ORDER:
                    for ic in range(NIC):
                        for bt in range(BT):
                            col = ic * BC + bt * P
                            nc.tensor.matmul(
                                psums[bt][:],
                                gr[k][:, col : col + P],
                                r_t[k - 1][:, ic * O : (ic + 1) * O],
                                start=False,
                                stop=False,
                            )
                else:
                    # last block: finish b-tiles one at a time so the psum
                    # evictions/stores overlap the remaining matmuls
                    for bt in range(BT):
                        for ic in range(NIC):
                            col = ic * BC + bt * P
                            nc.tensor.matmul(
                                psums[bt][:],
                                gr[k][:, col : col + P],
                                r_t[k - 1][:, ic * O : (ic + 1) * O],
                                start=False,
                                stop=ic == NIC - 1,
                            )
                        dst = out_t[:, bt * O : (bt + 1) * O]
                        if bt % 2 == 0:
                            nc.scalar.copy(dst, psums[bt][:])
                        else:
                            nc.vector.tensor_copy(dst, psums[bt][:])
                        nc.sync.dma_start(
                            out_d[:, bt * O : (bt + 1) * O],
                            out_t[:, bt * O : (bt + 1) * O],
                        )
    nc.compile()
    return nc


def _prep_operands(weights, coeff):
    """Host-side, input-independent preprocessing of the layer constants."""
    c, _ = _consts()
    Cw = coeff.astype(np.float64) * weights.astype(np.float64)[:, :, None]
    bias = Cw[:, :, 0].sum(axis=1)                      # [O]
    r = np.empty((ORDER, P, FREE), dtype=np.float32)
    for k in range(1, ORDER + 1):
        tmp = (Cw[:, :, k] / c[k]).T.astype(np.float32)  # [I, O]
        r[k - 1] = tmp.reshape(NIC, P, O).transpose(1, 0, 2).reshape(P, FREE)
    r = np.ascontiguousarray(
        r.transpose(1, 0, 2).reshape(P, ORDER * FREE)
    ).astype(np.float16)
    consts = np.ones((P, P + O), dtype=np.float32)
    consts[0, P:] = bias
    consts[1:, :] = 0.5
    return r, consts.astype(np.float16)


def _prep_x(x):
    """Per-core [128, FREE] views of x^T: xt[p, ic*BC + b] = x[core*BC+b, ic*128+p]."""
    shards = []
    for core in range(NCORES):
        xc = np.ascontiguousarray(x[core * BC : (core + 1) * BC, :].T)  # [I, BC]
        flat = xc.reshape(NIC, P, BC).transpose(1, 0, 2).reshape(P, FREE)
        shards.append(
            np.ascontiguousarray(
                flat.reshape(P, 2, FREE // 2).transpose(1, 0, 2)
            )
        )
    return shards


def _install_ntff_hook():
    """Register the NTFF profile hook that the image's boot skips (no
    antenv.axon_hooks module). Same ctypes ABI as trn_boot's
    _ntff_profile_via_ctypes. Only used for traced (profiling) runs."""
    import sys
    import types
    import ctypes
    import contextlib

    if "antenv.axon_hooks" in sys.modules:
        return
    mod = types.ModuleType("antenv.axon_hooks")
    state = {"hook": None}
    mod.set_axon_ntff_profile_hook = lambda h: state.__setitem__("hook", h)
    mod.get_axon_ntff_profile_hook = lambda: state["hook"]
    sys.modules["antenv.axon_hooks"] = mod
    import antenv

    antenv.axon_hooks = mod

    so_path = "/opt/axon/libaxon_pjrt.so"
    lib = ctypes.CDLL(so_path)
    if not hasattr(lib, "axon_start_nrt_profile"):
        return
    lib.axon_start_nrt_profile.argtypes = [
        ctypes.POINTER(ctypes.c_int64),
        ctypes.c_size_t,
    ]
    lib.axon_start_nrt_profile.restype = ctypes.c_int64
    lib.axon_stop_nrt_profile.argtypes = [ctypes.c_char_p]
    lib.axon_stop_nrt_profile.restype = ctypes.c_int64

    @contextlib.contextmanager
    def _hook(output_dir, device_ids):
        import jax

        jax.devices()
        if device_ids:
            ids = (ctypes.c_int64 * len(device_ids))(*device_ids)
            rc = lib.axon_start_nrt_profile(ids, len(device_ids))
        else:
            rc = lib.axon_start_nrt_profile(None, 0)
        if rc != 0:
            raise RuntimeError(f"axon_start_nrt_profile rc={rc}")
        try:
            yield
        finally:
            n = lib.axon_stop_nrt_profile(str(output_dir).encode())
            print(f"ntff profile: {n} file(s) written to {output_dir}")

    mod.set_axon_ntff_profile_hook(_hook)


_NC_CACHE = None


def _get_module():
    global _NC_CACHE
    if _NC_CACHE is None:
        _NC_CACHE = _build_module()
    return _NC_CACHE


def _run(x, weights, coeff, trace=False):
    nc = _get_module()
    r, consts = _prep_operands(weights, coeff)
    xs = _prep_x(np.asarray(x, dtype=np.float32))
    in_maps = [
        {"xt": xs[core], "r": r, "consts": consts} for core in range(NCORES)
    ]
    try:
        res = run_bass_kernel_spmd(
            nc, in_maps, core_ids=list(range(NCORES)), trace=trace
        )
    except Exception:
        res = run_bass_kernel_spmd(
            nc, in_maps, core_ids=list(range(NCORES)), trace=trace
        )
    out = np.concatenate(
        [
            res.results[core]["out"]
            .reshape(P, BT, O)
            .transpose(1, 0, 2)
            .reshape(BC, O)
            for core in range(NCORES)
        ],
        axis=0,
    )
    return out, res


def kernel(x, weights, coeff):
    out, _ = _run(x, weights, coeff, trace=False)
    return out


def kernel_traced(x, weights, coeff):
    _install_ntff_hook()
    out, res = _run(x, weights, coeff, trace=True)
    return out, res



# revision 3
# speedup vs baseline: 1.1767x; 1.1767x over previous
"""Trainium2 Bass kernel for a custom Jacobi-basis layer.

Math:
    t = tanh(x)                                  x: [B, I] f32
    J[b,i,k] = P_k^(1,1)(t[b,i])                 Jacobi polys, k = 0..8
    out[b,o] = sum_{i,k} J[b,i,k] * coeff[o,i,k] * weights[o,i]

Strategy (8 NeuronCores, data-parallel over batch):
  * The matmul only needs SOME degree-graded polynomial basis of t, not the
    Jacobi planes themselves: the change of basis is folded into the host-
    prepared operand rho = Cw @ T (Cw[o,i,k] = coeff*weights, T maps Jacobi
    onto the device basis).  Device basis (all fp16):
        V1 = t            V2 = t*t (=s)     V3 = t*M0    V4 = M0*M1
        V5 = t*V4         V6 = M2*V4        V7 = t*V6    V8 = M3*V6
    with Mj = alpha_j*(s - gamma_j) (shifted squares, conditioning knobs).
    V3..V8 are pure tensor_tensor products, which run at 2x on DVE in fp16;
    Mj are single scalar-engine Copy activations (scale+bias).  This takes
    the basis generation far off the critical path (DVE ~11us, Scalar ~8us
    vs PE ~29us), unlike the exact fp32 recurrence (~30us on each).
  * k=0 (J_0 == 1) becomes a bias, applied with K=1 matmuls ones(8.0) x
    (bias/8) mid-stream (full-clock region, no early PE cost).
  * All input DMAs are enqueued dependency-free: xt + outs on the Sync
    hardware queue, r planes + bias on the GpSimd queue, so descriptors
    stream back-to-back at wire speed instead of the ~2us/plane issue
    round-trips a completion-chained ladder costs.
  * x ships as fp16 (error contribution measured ~3e-5); out is DMA'd
    straight from PSUM.  Junk matmuls on a memset tile warm the PE clock
    before the stream and keep it high through the NEFF teardown ladder.
"""

import numpy as np

import concourse.mybir as mybir
import concourse.tile as tile
from concourse import bacc
from concourse.bass_utils import run_bass_kernel_spmd

ORDER = 8
B, I, O = 4096, 512, 512
NCORES = 8
BC = B // NCORES          # batch rows per core = 512
P = 128                   # partitions
NIC = I // P              # i-chunks = 4
BT = BC // P              # b-tiles per core = 4
FREE = NIC * BC           # free dim of basis planes = 2048
H = FREE // 2

GAMMA = (0.0615, 0.23, 0.47, 0.73)
ALPHA = tuple(1.0 / max(g, 1.0 - g) for g in GAMMA)
N_WARM = 5
N_TAIL = 10


def _jacobi_t(t, order=ORDER, a=1.0, b=1.0):
    vals = [np.ones_like(t), 0.5 * (a + b + 2) * t - 0.5 * (a - b)]
    for i in range(2, order + 1):
        k1 = (2 * i + a + b) * (2 * i + a + b - 1) / (2 * i * (i + a + b))
        k3 = (i + a - 1) * (i + b - 1) * (2 * i + a + b) / (
            i * (i + a + b) * (2 * i + a + b - 2)
        )
        vals.append(k1 * t * vals[-1] - k3 * vals[-2])
    return np.stack(vals, axis=0)  # [order+1, n]


def _basis_transform():
    """T[k,l] with J_k(t) = sum_l T[k,l] V_l(t); V_0 = 1."""
    t = np.linspace(-0.99999, 0.99999, 4001)
    s = t * t
    M = [ALPHA[j] * (s - GAMMA[j]) for j in range(4)]
    V = np.stack(
        [
            np.ones_like(t), t, s, t * M[0], M[0] * M[1], t * M[0] * M[1],
            M[0] * M[1] * M[2], t * M[0] * M[1] * M[2],
            M[0] * M[1] * M[2] * M[3],
        ],
        axis=0,
    )
    J = _jacobi_t(t)
    return J @ np.linalg.pinv(V)


def _build_module():
    nc = bacc.Bacc("TRN2", num_devices=NCORES)
    f32 = mybir.dt.float32
    f16 = mybir.dt.float16
    mult = mybir.AluOpType.mult
    subtract = mybir.AluOpType.subtract

    # xt half-major: [h, p, H], fp16, row = 2KB contiguous
    xt_d = nc.dram_tensor("xt", [2, P, H], f16, kind="ExternalInput")
    # r plane-major: [l, p, ic*O + o] = rho[o, ic*128+p, l+1], contiguous/plane
    r_d = nc.dram_tensor("r", [ORDER, P, FREE], f16, kind="ExternalInput")
    # bias/8 row (ones operand is memset to 8.0)
    bias_d = nc.dram_tensor("biasrow", [1, O], f16, kind="ExternalInput")
    # out[bt, p, o] = output[core*BC + bt*128 + p, o]
    out_d = nc.dram_tensor("out", [BT, P, O], f32, kind="ExternalOutput")

    with tile.TileContext(nc) as tc:
        with (
            tc.tile_pool(name="sb", bufs=1) as sb,
            tc.tile_pool(name="psum", bufs=1, space="PSUM") as pp,
        ):
            # --- DMA enqueue: no inter-DMA deps; two hardware queues ---
            xt_t = sb.tile([P, FREE], f16, tag="xt")
            nc.sync.dma_start(xt_t[:, 0:H], xt_d[0])
            nc.sync.dma_start(xt_t[:, H:FREE], xt_d[1])
            r_t = [sb.tile([P, FREE], f16, tag=f"r{l}", name=f"r{l}") for l in range(ORDER)]
            nc.gpsimd.dma_start(r_t[0][:, 0:H], r_d[0, :, 0:H])
            nc.gpsimd.dma_start(r_t[0][:, H:FREE], r_d[0, :, H:FREE])
            for l in range(1, ORDER):
                nc.gpsimd.dma_start(r_t[l][:], r_d[l])
            bias_t = sb.tile([1, O], f16, tag="bias")
            nc.gpsimd.dma_start(bias_t[:], bias_d[:])

            # --- PE warmup (memset-gated only) ---
            warm_t = sb.tile([P, P + O], f16, tag="warm")
            nc.vector.memset(warm_t[:], 0.25)
            ones_t = sb.tile([1, P], f16, tag="ones")
            nc.vector.memset(ones_t[:], 8.0)
            ps_warm = pp.tile([P, O], f32, tag="warmps", name="ps_warm")
            for _ in range(N_WARM):
                nc.tensor.matmul(
                    ps_warm[:], warm_t[:, 0:P], warm_t[:, P : P + O],
                    start=True, stop=True,
                )

            # --- basis planes ---
            t_t = sb.tile([P, FREE], f16, tag="t")
            s_t = sb.tile([P, FREE], f16, tag="s")
            m_t = [sb.tile([P, FREE], f16, tag=f"m{j}", name=f"m{j}") for j in range(4)]
            v_t = [sb.tile([P, FREE], f16, tag=f"v{l}", name=f"v{l}") for l in range(6)]
            halves = (slice(0, H), slice(H, FREE))
            Tanh = mybir.ActivationFunctionType.Tanh
            Copy = mybir.ActivationFunctionType.Copy

            # Scalar: tanh halves, M0 halves, M1, M2
            for h in (0, 1):
                nc.scalar.activation(t_t[:, halves[h]], xt_t[:, halves[h]], Tanh)
            # DVE: s halves (fp16 TT at 2x)
            for h in (0, 1):
                nc.vector.tensor_tensor(
                    s_t[:, halves[h]], t_t[:, halves[h]], t_t[:, halves[h]], mult
                )
            for h in (0, 1):
                nc.scalar.activation(
                    m_t[0][:, halves[h]], s_t[:, halves[h]], Copy,
                    bias=float(-ALPHA[0] * GAMMA[0]), scale=float(ALPHA[0]),
                )
            nc.scalar.activation(
                m_t[1][:], s_t[:], Copy,
                bias=float(-ALPHA[1] * GAMMA[1]), scale=float(ALPHA[1]),
            )
            nc.scalar.activation(
                m_t[2][:], s_t[:], Copy,
                bias=float(-ALPHA[2] * GAMMA[2]), scale=float(ALPHA[2]),
            )
            # DVE: V3..V8 as TT products; M3 via tensor_scalar
            nc.vector.tensor_tensor(v_t[0][:], t_t[:], m_t[0][:], mult)      # V3
            nc.vector.tensor_tensor(v_t[1][:], m_t[0][:], m_t[1][:], mult)   # V4
            nc.vector.tensor_tensor(v_t[2][:], t_t[:], v_t[1][:], mult)      # V5
            nc.vector.tensor_scalar(                                         # M3
                m_t[3][:], s_t[:], float(ALPHA[3]), float(ALPHA[3] * GAMMA[3]),
                mult, subtract,
            )
            nc.vector.tensor_tensor(v_t[3][:], m_t[2][:], v_t[1][:], mult)   # V6
            nc.vector.tensor_tensor(v_t[4][:], t_t[:], v_t[3][:], mult)      # V7
            nc.vector.tensor_tensor(v_t[5][:], m_t[3][:], v_t[3][:], mult)   # V8

            planes = [t_t, s_t] + v_t                                        # V1..V8

            # --- matmul stream ---
            psums = [
                pp.tile([P, O], f32, tag=f"ps{bt}", name=f"ps{bt}")
                for bt in range(BT)
            ]
            for l in range(ORDER - 1):
                for ic in range(NIC):
                    for bt in range(BT):
                        col = ic * BC + bt * P
                        nc.tensor.matmul(
                            psums[bt][:],
                            planes[l][:, col : col + P],
                            r_t[l][:, ic * O : (ic + 1) * O],
                            start=(l == 0 and ic == 0),
                            stop=False,
                        )
                if l == 3:
                    # bias: ones(8) x (bias/8), K=1, mid-stream (full clock)
                    for bt in range(BT):
                        nc.tensor.matmul(
                            psums[bt][:], ones_t[:], bias_t[:],
                            start=False, stop=False,
                        )
            # last plane bt-major: finish banks one at a time, evict + DMA
            out_t = sb.tile([P, BT * O], f32, tag="out")
            l = ORDER - 1
            for bt in range(BT):
                for ic in range(NIC):
                    col = ic * BC + bt * P
                    nc.tensor.matmul(
                        psums[bt][:],
                        planes[l][:, col : col + P],
                        r_t[l][:, ic * O : (ic + 1) * O],
                        start=False,
                        stop=ic == NIC - 1,
                    )
                dst = out_t[:, bt * O : (bt + 1) * O]
                if bt % 2 == 0:
                    nc.scalar.copy(dst, psums[bt][:])
                else:
                    nc.vector.tensor_copy(dst, psums[bt][:])
                nc.sync.dma_start(out_d[bt], dst)
            # keep the PE clock high through the teardown ladder
            for _ in range(N_TAIL):
                nc.tensor.matmul(
                    ps_warm[:], warm_t[:, 0:P], warm_t[:, P : P + O],
                    start=True, stop=True,
                )
    nc.compile()
    return nc


def _prep_operands(weights, coeff):
    """Host-side, input-independent preprocessing of the layer constants."""
    T = _basis_transform()
    Cw = coeff.astype(np.float64) * weights.astype(np.float64)[:, :, None]
    rho = np.einsum("oik,kl->oil", Cw, T)
    bias8 = (rho[:, :, 0].sum(axis=1) / 8.0).astype(np.float16)[None, :]
    r = np.empty((ORDER, P, FREE), dtype=np.float16)
    for l in range(1, ORDER + 1):
        tmp = rho[:, :, l].T.astype(np.float32)          # [I, O]
        r[l - 1] = tmp.reshape(NIC, P, O).transpose(1, 0, 2).reshape(P, FREE)
    return np.ascontiguousarray(r), np.ascontiguousarray(bias8)


def _prep_x(x):
    """Per-core [2, 128, H] fp16 views of x^T: col c = ic*BC + b."""
    shards = []
    for core in range(NCORES):
        xc = np.ascontiguousarray(x[core * BC : (core + 1) * BC, :].T)  # [I, BC]
        flat = xc.reshape(NIC, P, BC).transpose(1, 0, 2).reshape(P, FREE)
        shards.append(
            np.ascontiguousarray(
                flat.reshape(P, 2, H).transpose(1, 0, 2)
            ).astype(np.float16)
        )
    return shards


def _install_ntff_hook():
    """Register the NTFF profile hook that the image's boot skips (no
    antenv.axon_hooks module). Same ctypes ABI as trn_boot's
    _ntff_profile_via_ctypes. Only used for traced (profiling) runs."""
    import sys
    import types
    import ctypes
    import contextlib

    if "antenv.axon_hooks" in sys.modules:
        return
    mod = types.ModuleType("antenv.axon_hooks")
    state = {"hook": None}
    mod.set_axon_ntff_profile_hook = lambda h: state.__setitem__("hook", h)
    mod.get_axon_ntff_profile_hook = lambda: state["hook"]
    sys.modules["antenv.axon_hooks"] = mod
    import antenv

    antenv.axon_hooks = mod

    so_path = "/opt/axon/libaxon_pjrt.so"
    lib = ctypes.CDLL(so_path)
    if not hasattr(lib, "axon_start_nrt_profile"):
        return
    lib.axon_start_nrt_profile.argtypes = [
        ctypes.POINTER(ctypes.c_int64),
        ctypes.c_size_t,
    ]
    lib.axon_start_nrt_profile.restype = ctypes.c_int64
    lib.axon_stop_nrt_profile.argtypes = [ctypes.c_char_p]
    lib.axon_stop_nrt_profile.restype = ctypes.c_int64

    @contextlib.contextmanager
    def _hook(output_dir, device_ids):
        import jax

        jax.devices()
        if device_ids:
            ids = (ctypes.c_int64 * len(device_ids))(*device_ids)
            rc = lib.axon_start_nrt_profile(ids, len(device_ids))
        else:
            rc = lib.axon_start_nrt_profile(None, 0)
        if rc != 0:
            raise RuntimeError(f"axon_start_nrt_profile rc={rc}")
        try:
            yield
        finally:
            n = lib.axon_stop_nrt_profile(str(output_dir).encode())
            print(f"ntff profile: {n} file(s) written to {output_dir}")

    mod.set_axon_ntff_profile_hook(_hook)


_NC_CACHE = None


def _get_module():
    global _NC_CACHE
    if _NC_CACHE is None:
        _NC_CACHE = _build_module()
    return _NC_CACHE


def _run(x, weights, coeff, trace=False):
    nc = _get_module()
    r, bias8 = _prep_operands(weights, coeff)
    xs = _prep_x(np.asarray(x, dtype=np.float32))
    in_maps = [
        {"xt": xs[core], "r": r, "biasrow": bias8} for core in range(NCORES)
    ]
    try:
        res = run_bass_kernel_spmd(
            nc, in_maps, core_ids=list(range(NCORES)), trace=trace
        )
    except Exception:
        res = run_bass_kernel_spmd(
            nc, in_maps, core_ids=list(range(NCORES)), trace=trace
        )
    out = np.concatenate(
        [res.results[core]["out"].reshape(BC, O) for core in range(NCORES)],
        axis=0,
    )
    return out, res


def kernel(x, weights, coeff):
    out, _ = _run(x, weights, coeff, trace=False)
    return out


def kernel_traced(x, weights, coeff):
    _install_ntff_hook()
    out, res = _run(x, weights, coeff, trace=True)
    return out, res
